# revision 3
# baseline (speedup 1.0000x reference)
"""Trainium2 Bass kernel for nn_CustomLoss_62079457296845.

Computes L = mean((y_hat - y)^2) + mean((y_hat - mag4c)^2) where
y_hat = (mag4uc - rowdot(A, beta + c) - y_mean) / y_scale, over
N=4194304 rows, D=18 features.

Strategy: pure data parallel over 8 NeuronCores; each core streams its
524288-row shard through SBUF in [128 x W x 18] tiles. Per tile (DVE):
  bc   = c + beta      (tensor_tensor add, beta materialized per-tile via
                        a broadcast SBUF constant)
  prod = A * bc        (tensor_tensor mult, in place)
  rd   = reduce_sum(prod, axis=-1)           # row dots
  h    = mag4uc - rd                         (scalar_tensor_tensor)
  t1   = s*h - y ; t2 = s*h - mag4c          (scalar_tensor_tensor)
ScalarE: sq/accumulate via activation(Square, bias=-y_mean*s, accum_out)
per-core output: [128, 2*NT] per-tile partial sums; host sums in f64.
"""

import os
import sys

import numpy as np

for _p in ("/opt/trn_rl_repo",):
    if _p not in sys.path and os.path.isdir(_p):
        sys.path.insert(0, _p)

N = 4194304
D = 18
NCORES = 8
R = N // NCORES          # rows per core
P = 128                  # SBUF partitions
W = 256                  # rows per partition per tile
NT = R // (P * W)        # tiles per core (16)
TW = W * D               # free elems per partition for A/C tiles


def _build(s: float, bg: float):
    """Build the Bass program. s = 1/y_scale, bg = -y_mean/y_scale."""
    from contextlib import ExitStack

    import concourse.bass as bass
    import concourse.tile as tile
    from concourse import bacc, mybir

    f32 = mybir.dt.float32
    Alu = mybir.AluOpType

    nc = bacc.Bacc("TRN2", debug=False, target_bir_lowering=False,
                   num_devices=NCORES)

    A_d = nc.dram_tensor("A_t", [NT, P, W, D], f32, kind="ExternalInput").ap()
    C_d = nc.dram_tensor("C_t", [NT, P, W, D], f32, kind="ExternalInput").ap()
    Y_d = nc.dram_tensor("Y_t", [NT, P, W], f32, kind="ExternalInput").ap()
    U_d = nc.dram_tensor("U_t", [NT, P, W], f32, kind="ExternalInput").ap()
    M_d = nc.dram_tensor("M_t", [NT, P, W], f32, kind="ExternalInput").ap()
    B_d = nc.dram_tensor("B_rep", [1, TW], f32, kind="ExternalInput").ap()
    out_d = nc.dram_tensor("out", [P, 2 * NT], f32, kind="ExternalOutput").ap()

    with ExitStack() as ctx:
        tc = ctx.enter_context(tile.TileContext(nc))
        consts = ctx.enter_context(tc.tile_pool(name="consts", bufs=1))
        big = ctx.enter_context(tc.tile_pool(name="big", bufs=3))
        small = ctx.enter_context(tc.tile_pool(name="small", bufs=4))

        beta_sb = consts.tile([P, W, D], f32)
        nc.sync.dma_start(out=beta_sb, in_=B_d.to_broadcast((P, TW)))

        bias_sb = consts.tile([P, 1], f32)
        nc.vector.memset(bias_sb, float(bg))

        outs = consts.tile([P, 2 * NT], f32)

        for i in range(NT):
            a = big.tile([P, W, D], f32, tag="a")
            nc.sync.dma_start(out=a, in_=A_d[i])
            c = big.tile([P, W, D], f32, tag="c")
            nc.sync.dma_start(out=c, in_=C_d[i])
            y = small.tile([P, W], f32, tag="y")
            nc.sync.dma_start(out=y, in_=Y_d[i])
            u = small.tile([P, W], f32, tag="u")
            nc.sync.dma_start(out=u, in_=U_d[i])
            m = small.tile([P, W], f32, tag="m")
            nc.sync.dma_start(out=m, in_=M_d[i])

            nc.vector.tensor_tensor(out=c, in0=c, in1=beta_sb, op=Alu.add)
            nc.vector.tensor_tensor(out=c, in0=a, in1=c, op=Alu.mult)
            rd = small.tile([P, W], f32, tag="rd")
            nc.vector.tensor_reduce(out=rd, in_=c, axis=mybir.AxisListType.X,
                                    op=Alu.add)
            h = small.tile([P, W], f32, tag="h")
            nc.vector.scalar_tensor_tensor(out=h, in0=rd, scalar=-1.0,
                                           in1=u, op0=Alu.mult, op1=Alu.add)
            t1 = small.tile([P, W], f32, tag="t1")
            nc.vector.scalar_tensor_tensor(out=t1, in0=h, scalar=float(s),
                                           in1=y, op0=Alu.mult,
                                           op1=Alu.subtract)
            t2 = small.tile([P, W], f32, tag="t2")
            nc.vector.scalar_tensor_tensor(out=t2, in0=h, scalar=float(s),
                                           in1=m, op0=Alu.mult,
                                           op1=Alu.subtract)
            sq1 = small.tile([P, W], f32, tag="sq1")
            nc.scalar.activation(out=sq1, in_=t1,
                                 func=mybir.ActivationFunctionType.Square,
                                 bias=bias_sb[:], scale=1.0,
                                 accum_out=outs[:, 2 * i:2 * i + 1])
            sq2 = small.tile([P, W], f32, tag="sq2")
            nc.scalar.activation(out=sq2, in_=t2,
                                 func=mybir.ActivationFunctionType.Square,
                                 bias=bias_sb[:], scale=1.0,
                                 accum_out=outs[:, 2 * i + 1:2 * i + 2])

        nc.sync.dma_start(out=out_d, in_=outs)

    nc.compile()
    return nc


def _shard_inputs(c, y, A, mag4uc, mag4c, beta):
    beta_rep = np.ascontiguousarray(
        np.tile(np.asarray(beta, np.float32).reshape(D), W).reshape(1, TW))
    in_maps = []
    for k in range(NCORES):
        lo, hi = k * R, (k + 1) * R
        in_maps.append({
            "A_t": np.ascontiguousarray(
                np.asarray(A[lo:hi], np.float32).reshape(NT, P, W, D)),
            "C_t": np.ascontiguousarray(
                np.asarray(c[lo:hi], np.float32).reshape(NT, P, W, D)),
            "Y_t": np.ascontiguousarray(
                np.asarray(y[lo:hi], np.float32).reshape(NT, P, W)),
            "U_t": np.ascontiguousarray(
                np.asarray(mag4uc[lo:hi], np.float32).reshape(NT, P, W)),
            "M_t": np.ascontiguousarray(
                np.asarray(mag4c[lo:hi], np.float32).reshape(NT, P, W)),
            "B_rep": beta_rep,
        })
    return in_maps


def _run(inputs: dict, trace: bool = False):
    from concourse.bass_utils import run_bass_kernel_spmd

    y_scale = float(np.asarray(inputs["y_scale"]).reshape(-1)[0])
    y_mean = float(np.asarray(inputs["y_mean"]).reshape(-1)[0])
    s = 1.0 / y_scale
    bg = -y_mean * s

    nc = _build(s, bg)
    in_maps = _shard_inputs(inputs["c"], inputs["y"], inputs["A"],
                            inputs["mag4uc"], inputs["mag4c"], inputs["beta"])
    res = run_bass_kernel_spmd(nc, in_maps, list(range(NCORES)), trace=trace)
    total = np.float64(0.0)
    for r in res.results:
        total += r["out"].astype(np.float64).sum()
    loss = np.float32(total / N)
    return np.asarray(loss, dtype=np.float32), res


def kernel(**inputs) -> np.ndarray:
    out, _ = _run(inputs, trace=False)
    return out
